# revision 5
# baseline (speedup 1.0000x reference)
"""CARAFE + MSGConv Trainium2 kernel v2 (8 NeuronCores, spatial x batch sharding).

out[c, i, j] = sum_{p,q} W[5p+q, i, j] * Xpad[c, i//2 + p - 2, j//2 + q - 2]

Per core: one batch element (core//4), a 16-source-row block (core%4).
v2 changes vs baseline:
 - depthwise tap MACs split across DVE and GPSIMD (Pool) engines
 - enc dw repacked into two [128, 272]-wide slabs, 32-aligned row quarters
   (dw3 runs only its 9 taps)
 - dw outputs consumed in packed form by the next 1x1 conv via K-split
   accumulating matmuls at legal base partitions {0,32,64}; only the
   4th group needs a fix-up DMA to a base-32/64 scratch block
 - encoder px computed transposed ([100 sk, 1024 pix]); softmax via
   matmul column-sums + reciprocal + replicate-matmul + elementwise mult
 - backend: transpose+replicate fused into one matmul per (t,jb) using a
   broadcast access pattern on lhsT; batched PSUM->SBUF copies; bf16 out
"""

import sys

sys.path.insert(0, "/opt/trn_rl_repo")

from contextlib import ExitStack

import ml_dtypes
import numpy as np

import concourse.bass as bass
import concourse.tile as tile
from concourse import bacc, library_config, mybir
from concourse.ap import AP
from concourse.bass_utils import run_bass_kernel_spmd

BF16 = mybir.dt.bfloat16
F32 = mybir.dt.float32
I16 = mybir.dt.int16
AF = mybir.ActivationFunctionType
OP = mybir.AluOpType
nbf = ml_dtypes.bfloat16

C = 128
H = W = 64
NCORES = 8
XR = 24          # X shard rows (16 + 4 halo each side)
XW = 68          # padded slab pitch
NEG = -30.0      # additive pre-activation mask; SiLU(-30) ~= -2.8e-12

# packa column layout (bf16 [128, 904])
PA_CV1 = 0       # w_cv1 [128, 32]
PA_PXLO = 32     # w_px rows 0:32  [32, 64]
PA_PXHI = 96     # w_px rows 32:64, replicated at bases 0/32/64 [96, 64]
PA_ECV1 = 160    # w_ecv1 [65, 50]
PA_EPXA = 210    # w_epx rows 0:50 [50, 100]
PA_EPXB = 310    # w_epx rows 50:75, replicated at bases 0/32/64 [89, 100]
PA_EPXC = 410    # w_epx rows 75:100, replicated at bases 0/32/64 [89, 100]
PA_ID = 510      # identity [128, 128]
PA_ESEL = 638    # esel [100, 4]
PA_ESELT = 642   # esel4T [4, 100]
PA_ONES = 742    # ones [1, 32]
PA_EONES = 774   # ones [1, 50]
PA_WIDTH = 832

# packb column layout (f32 [128, 66])
PB_WDWP = 0      # comp dw taps [128, 25]
PB_WE3 = 25      # enc dw3 taps [128, 9]  (32-aligned quarters)
PB_WE5 = 34      # enc dw5 taps [128, 25]
PB_BDWP = 59     # comp dw bias [128, 1]
PB_BE3 = 60      # enc dw3 bias [128, 1]
PB_BE5 = 61      # enc dw5 bias [128, 1]
PB_BCV1 = 62     # [32, 1]
PB_BPX = 63      # [64, 1]
PB_BECV1 = 64    # [50, 1]
PB_BEPX = 65     # [100, 1]
PB_WIDTH = 66


# ======================================================================
# host-side parameter prep
# ======================================================================

def _fold_1x1(w, s):
    return (w[:, :, 0, 0] * s[:, None]).T.copy()


def _dw_taps(w, s, k):
    ch = w.shape[0]
    out = np.zeros((ch, 25), np.float32)
    off = (5 - k) // 2
    for ty in range(k):
        for tx in range(k):
            out[:, 5 * (ty + off) + (tx + off)] = w[:, 0, ty, tx] * s
    return out


def _pad32(a):
    """[25ch x 4q] tap table -> [128] with 32-aligned quarters."""
    out = np.zeros((128,) + a.shape[1:], a.dtype)
    for q in range(4):
        out[32 * q:32 * q + 25] = a[25 * q:25 * q + 25]
    return out


def _host_consts(inputs):
    d = {}
    w_cv1 = _fold_1x1(inputs["comp_cv1_w"], inputs["comp_cv1_s"])     # [128,32]
    w3 = _dw_taps(inputs["comp_dw3_w"], inputs["comp_dw3_s"], 3)
    w5 = _dw_taps(inputs["comp_dw5_w"], inputs["comp_dw5_s"], 5)
    w_dwp = np.tile(np.concatenate([w3, w5], 0), (4, 1))              # [128,25]
    b_dwp = np.tile(
        np.concatenate([inputs["comp_dw3_b"], inputs["comp_dw5_b"]]), 4
    ).reshape(128, 1)
    w_px = _fold_1x1(inputs["comp_px_w"], inputs["comp_px_s"])        # [64,64]
    w_ecv1 = _fold_1x1(inputs["enc_cv1_w"], inputs["enc_cv1_s"])      # [64,50]
    # enc dw taps in [25ch x 4 row-quarters] -> padded to 32-aligned
    e3 = inputs["enc_dw3_w"][:, 0] * inputs["enc_dw3_s"][:, None, None]
    e5 = inputs["enc_dw5_w"][:, 0] * inputs["enc_dw5_s"][:, None, None]
    w_e3 = _pad32(np.tile(e3.reshape(25, 9), (4, 1)))                 # [128,9]
    w_e5 = _pad32(np.tile(e5.reshape(25, 25), (4, 1)))                # [128,25]
    b_e3 = _pad32(np.tile(inputs["enc_dw3_b"], 4).reshape(100, 1))
    b_e5 = _pad32(np.tile(inputs["enc_dw5_b"], 4).reshape(100, 1))
    w_epx = _fold_1x1(inputs["enc_px_w"], inputs["enc_px_s"])         # [100,100]

    pa = np.zeros((128, PA_WIDTH), np.float32)
    pa[0:128, PA_CV1:PA_CV1 + 32] = w_cv1
    pa[0:64, PA_PXLO:PA_PXLO + 64] = w_px
    pa[0:64, PA_ECV1:PA_ECV1 + 50] = w_ecv1
    pa[0:100, PA_EPXA:PA_EPXA + 100] = w_epx
    # K-split weights replicated at bases 0/32/64 so lhsT base partition
    # matches the packed rhs slab's base partition
    for b0 in (0, 32, 64):
        pa[b0:b0 + 32, PA_PXHI:PA_PXHI + 64] = w_px[32:64]
        pa[b0:b0 + 25, PA_EPXB:PA_EPXB + 100] = w_epx[50:75]
        pa[b0:b0 + 25, PA_EPXC:PA_EPXC + 100] = w_epx[75:100]
    pa[0:128, PA_ID:PA_ID + 128] = np.eye(128)
    for s in range(4):
        for k in range(25):
            pa[4 * k + s, PA_ESEL + s] = 1.0                          # esel
            pa[s, PA_ESELT + 4 * k + s] = 1.0                         # esel4T
    pa[0, PA_ONES:PA_ONES + 32] = 1.0
    pa[0, PA_EONES:PA_EONES + 50] = 1.0
    d["packa"] = pa.astype(nbf)

    pb = np.zeros((128, PB_WIDTH), np.float32)
    pb[:, PB_WDWP:PB_WDWP + 25] = w_dwp
    pb[:, PB_WE3:PB_WE3 + 9] = w_e3
    pb[:, PB_WE5:PB_WE5 + 25] = w_e5
    pb[:, PB_BDWP:PB_BDWP + 1] = b_dwp
    pb[:, PB_BE3:PB_BE3 + 1] = b_e3
    pb[:, PB_BE5:PB_BE5 + 1] = b_e5
    pb[0:32, PB_BCV1:PB_BCV1 + 1] = inputs["comp_cv1_b"].reshape(32, 1)
    pb[0:64, PB_BPX:PB_BPX + 1] = inputs["comp_px_b"].reshape(64, 1)
    pb[0:50, PB_BECV1:PB_BECV1 + 1] = inputs["enc_cv1_b"].reshape(50, 1)
    pb[0:100, PB_BEPX:PB_BEPX + 1] = inputs["enc_px_b"].reshape(100, 1)
    d["packb"] = pb

    # wdiag [128, 30*128]: diagonal tap weights for PE-side dw taps
    # cols 0..14  -> comp taps 10..24 (w_dwp)
    # cols 15..29 -> enc dw5 taps 10..24 (w_e5)
    wd = np.zeros((128, 30 * 128), np.float32)
    for i in range(15):
        np.fill_diagonal(wd[:, 128 * i:128 * (i + 1)], w_dwp[:, 10 + i])
        np.fill_diagonal(wd[:, 128 * (15 + i):128 * (16 + i)], w_e5[:, 10 + i])
    d["wdiag"] = wd.astype(nbf)

    # repl [128, 4*128]: lhsT for the W row-replication matmul
    rp = np.zeros((128, 512), np.float32)
    for jb in range(4):
        for n in range(128):
            rho, j = divmod(n, 32)
            yl, xl = rho // 2, j // 2
            rp[64 * yl + 16 * jb + xl, 128 * jb + n] = 1.0
    d["repl"] = rp.astype(nbf)

    # sidx [128, 4*100] int16; horizontal out-of-image taps dropped (-1).
    si = np.full((128, 400), -1, np.int16)
    for n in range(128):
        rho, j = divmod(n, 32)
        yl, dy = divmod(rho, 2)
        xl, dx = divmod(j, 2)
        sn = 2 * dy + dx
        for jb in range(4):
            for cp in range(100):
                k, sc = divmod(cp, 4)
                if sc != sn:
                    continue
                p, q = divmod(k, 5)
                if not (0 <= 16 * jb + xl + q - 2 < 64):
                    continue
                si[n, 100 * jb + cp] = 120 * jb + 20 * (yl + p) + (xl + q)
    d["sidx"] = si
    return d


def _host_shard(X, core):
    b, ri = divmod(core, 4)
    r0 = 16 * ri - 4
    xs = np.zeros((C, XR, W), np.float32)
    lo, hi = max(0, r0), min(H, r0 + XR)
    xs[:, lo - r0 : hi - r0, :] = X[b, :, lo:hi, :]
    mrow = np.zeros((1, XR, W), np.float32)
    for r in range(XR):
        if not (0 <= r0 + r < H):
            mrow[0, r, :] = NEG
    emask = np.zeros((1, 20, W), np.float32)
    for r in range(20):
        if not (0 <= (16 * ri - 2) + r < H):
            emask[0, r, :] = NEG
    xsb = xs.astype(nbf)
    # pre-transposed X slabs, one [120, 128] per block (column-padded)
    xsp = np.zeros((C, XR, XW), nbf)
    xsp[:, :, 2 : 2 + W] = xsb
    xt = np.zeros((120, 32 * 128), nbf)
    for B in range(32):
        t, jb = divmod(B, 4)
        slab = xsp[:, 2 * t + 2 : 2 * t + 8, 16 * jb : 16 * jb + 20]
        xt[:, 128 * B : 128 * B + 128] = slab.reshape(C, 120).T
    mrowem = np.zeros((1, 2 * XR * W), np.float32)
    mrowem[0, 0 : XR * W] = mrow.reshape(XR * W)
    mrowem[0, XR * W : XR * W + 20 * W] = emask.reshape(20 * W)
    return (
        xsb.reshape(C, XR * W),
        mrowem.astype(nbf),
        xt,
    )


# ======================================================================
# device kernel
# ======================================================================

def build_kernel():
    nc = bacc.Bacc(
        "TRN2",
        target_bir_lowering=False,
        debug=False,
        enable_asserts=False,
        num_devices=NCORES,
    )

    def din(name, shape, dt):
        return nc.dram_tensor(name, list(shape), dt, kind="ExternalInput").ap()

    x_d = din("x", (128, XR * W), BF16)
    xt_d = din("xt", (120, 32 * 128), BF16)
    mrowem_d = din("mrowem", (1, 2 * XR * W), BF16)
    packa_d = din("packa", (128, PA_WIDTH), BF16)
    packb_d = din("packb", (128, PB_WIDTH), F32)
    repl_d = din("repl", (128, 512), BF16)
    wdiag_d = din("wdiag", (128, 30 * 128), BF16)
    sidx_d = din("sidx", (128, 400), I16)
    out_d = nc.dram_tensor("out", [128, 32 * 128], BF16, kind="ExternalOutput").ap()

    FS = 5 * XW                    # comp tap width 340
    FS2 = 4 * XW                   # enc tap width 272

    with tile.TileContext(nc) as tc, ExitStack() as ctx:
        cpool = ctx.enter_context(tc.tile_pool(name="consts", bufs=1))
        work = ctx.enter_context(tc.tile_pool(name="work", bufs=1))
        psF_cm = tc.tile_pool(name="psF", bufs=2, space="PSUM")
        psF = psF_cm.__enter__()
        psS_cm = tc.tile_pool(name="psS", bufs=1, space="PSUM")
        psS = psS_cm.__enter__()
        spool = ctx.enter_context(tc.tile_pool(name="stage", bufs=3))

        nc.gpsimd.load_library(library_config.local_scatter)

        packa = cpool.tile([128, PA_WIDTH], BF16, tag="packa")
        packb = cpool.tile([128, PB_WIDTH], F32, tag="packb")
        mrowem = cpool.tile([1, 2 * XR * W], BF16, tag="mrowem")
        xb = cpool.tile([128, XR * W], BF16, tag="x")
        xt = cpool.tile([120, 32 * 128], BF16, tag="xt")
        sidx = cpool.tile([128, 400], I16, tag="sidx")
        repl = cpool.tile([128, 512], BF16, tag="repl")
        wdiag = cpool.tile([128, 30 * 128], BF16, tag="wdiag")

        # sync queue: x first, then xt; scalar queue: weights + masks
        nc.sync.dma_start(xb[:], x_d)
        nc.scalar.dma_start(packa[:], packa_d)
        nc.scalar.dma_start(mrowem[:], mrowem_d)
        nc.sync.dma_start(xt[:], xt_d)
        nc.scalar.dma_start(packb[:], packb_d)
        nc.gpsimd.dma_start(sidx[:], sidx_d)
        nc.gpsimd.dma_start(repl[:], repl_d)
        nc.scalar.dma_start(wdiag[:], wdiag_d)

        w_cv1 = packa[0:128, PA_CV1:PA_CV1 + 32]
        w_px = packa[0:64, PA_PXLO:PA_PXLO + 64]
        w_ecv1 = packa[0:64, PA_ECV1:PA_ECV1 + 50]
        w_epx100 = packa[0:100, PA_EPXA:PA_EPXA + 100]
        ident = packa[0:128, PA_ID:PA_ID + 128]
        i100 = packa[0:100, PA_ID:PA_ID + 100]
        esel = packa[0:100, PA_ESEL:PA_ESEL + 4]
        esel4t = packa[0:4, PA_ESELT:PA_ESELT + 100]
        ones1 = packa[0:1, PA_ONES:PA_ONES + 32]
        eones = packa[0:1, PA_EONES:PA_EONES + 50]
        w_dwp = packb[0:128, PB_WDWP:PB_WDWP + 25]
        w_e3 = packb[0:128, PB_WE3:PB_WE3 + 9]
        w_e5 = packb[0:128, PB_WE5:PB_WE5 + 25]
        b_dwp = packb[0:128, PB_BDWP:PB_BDWP + 1]
        b_e3 = packb[0:128, PB_BE3:PB_BE3 + 1]
        b_e5 = packb[0:128, PB_BE5:PB_BE5 + 1]
        b_cv1 = packb[0:32, PB_BCV1:PB_BCV1 + 1]
        b_px = packb[0:64, PB_BPX:PB_BPX + 1]
        b_ecv1 = packb[0:50, PB_BECV1:PB_BECV1 + 1]
        b_epx = packb[0:100, PB_BEPX:PB_BEPX + 1]

        def pxhi(b0):
            return packa[b0:b0 + 32, PA_PXHI:PA_PXHI + 64]

        def epxb(b0):
            return packa[b0:b0 + 25, PA_EPXB:PA_EPXB + 100]

        def epxc(b0):
            return packa[b0:b0 + 25, PA_EPXC:PA_EPXC + 100]

        # ---- working tensors
        x12 = work.tile([64, XR * W], BF16)          # x1 (0:32) + x2 (32:64)
        enc_cat = work.tile([100, 16 * W], BF16)     # e1c + enc dw outs
        x1p = work.tile([128, 9 * XW + 8], BF16)     # packed x1 (68-pitch)
        x2p = work.tile([128, FS], BF16)             # comp dw out (packed)
        enc_in = work.tile([64, 20 * W], BF16)       # px out
        e1c = work.tile([50, 20 * W], BF16)          # enc cv1 out
        e3p = work.tile([128, 8 * XW + 8], BF16)     # enc dw3 slab
        e5p = work.tile([128, 8 * XW + 8], BF16)     # enc dw5 slab
        e3sil = work.tile([128, FS2], BF16)          # enc dw3 out (packed)
        e5sil = work.tile([128, FS2], BF16)          # enc dw5 out (packed)
        wts = work.tile([100, 1024], BF16)           # silu(enc px), transposed
        ev = work.tile([100, 1024], BF16)            # exp(wts)
        rv = work.tile([4, 1024], BF16)              # 1/colsum per subpixel
        wnorm = work.tile([100, 1024], BF16)         # softmaxed weights [sk,pix]
        # wcat_t: per-t transposed weights [pix-in-rowpair, sk]
        cA = work.tile([128, FS], BF16)
        cB = work.tile([128, FS], BF16)
        cC = work.tile([128, FS], BF16)
        cD = work.tile([128, FS], BF16)
        eA = work.tile([128, FS2], BF16)
        eB = work.tile([128, FS2], BF16)
        eC = work.tile([128, FS2], BF16)
        eF = work.tile([128, FS2], BF16)
        eG = work.tile([128, FS2], BF16)

        # warmup: trigger local_scatter ucode load early
        warm = work.tile([16, 16], BF16)
        nc.gpsimd.local_scatter(
            warm[:], packa[0:16, 0:2], sidx[:][0:16, 0:2],
            channels=16, num_elems=16, num_idxs=2,
        )
        # warmup: force silu act-table loads while inputs stream in
        wsin = work.tile([1, 2], BF16)
        wsout = work.tile([1, 2], BF16)
        nc.vector.memset(wsin[:], 0.0)
        nc.scalar.activation(wsout[:], wsin[:], AF.Silu)

        x123 = x12[:].rearrange("p (r c) -> p r c", c=W)
        enc_cat3 = enc_cat[:].rearrange("p (r c) -> p r c", c=W)
        e1c3 = e1c[:].rearrange("p (r c) -> p r c", c=W)
        x1p3 = x1p[:, 0 : 9 * XW].rearrange("p (r c) -> p r c", c=XW)
        e3p3 = e3p[:, 0 : 8 * XW].rearrange("p (r c) -> p r c", c=XW)
        e5p3 = e5p[:, 0 : 8 * XW].rearrange("p (r c) -> p r c", c=XW)

        # zero slab pad columns + pad partitions (25:32 per quarter)
        nc.vector.memset(x1p[:, 9 * XW : 9 * XW + 8], 0.0)
        nc.vector.memset(x1p3[:, :, 0:2], 0.0)
        nc.vector.memset(x1p3[:, :, 66:68], 0.0)
        nc.vector.memset(e3p[:], 0.0)
        nc.vector.memset(e5p[:], 0.0)

        # ---- comp cv1 + interleaved x1p packing
        for ch in range(3):
            ps = psF.tile([32, 512], F32, tag="convps")
            nc.tensor.matmul(
                ps[:], w_cv1, xb[:, 512 * ch : 512 * (ch + 1)],
                start=True, stop=False,
            )
            nc.tensor.matmul(
                ps[:], ones1, mrowem[0:1, 512 * ch : 512 * (ch + 1)],
                start=False, stop=True,
            )
            nc.scalar.activation(
                x12[0:32, 512 * ch : 512 * (ch + 1)], ps[:],
                AF.Silu, bias=b_cv1,
            )
            # pack row-groups as soon as their rows exist:
            # g0 needs rows 0:9 (ch0+ch1), g1 rows 5:14, g2 rows 10:19,
            # g3 rows 15:24 (ch2)
            packs = {1: (0, 1), 2: (2, 3)}.get(ch, ())
            for g in packs:
                eng = (nc.gpsimd, nc.gpsimd, nc.sync, nc.scalar)[g]
                eng.dma_start(
                    x1p3[32 * g : 32 * g + 32, 0:9, 2 : 2 + W],
                    x123[0:32, 5 * g : 5 * g + 9, :],
                )

        # ---- comp dw: 25 unified 5x5 taps, DVE (15) + Pool (10)
        def tap(eng, acc, slab, wsel, taps, offs, first):
            for i, t in enumerate(taps):
                sv = slab[:, offs[t] : offs[t] + (FS if slab is x1p else FS2)]
                if i == 0 and first:
                    eng.tensor_scalar(acc, sv, wsel[:, t : t + 1], None, OP.mult)
                else:
                    eng.scalar_tensor_tensor(
                        acc, sv, wsel[:, t : t + 1], acc, OP.mult, OP.add
                    )

        off5c = [(t // 5) * XW + (t % 5) for t in range(25)]
        psd = psF.tile([128, FS], F32, tag="dwps")
        for i in range(15):
            t = 10 + i
            nc.tensor.matmul(
                psd[:], wdiag[:, 128 * i : 128 * (i + 1)],
                x1p[:, off5c[t] : off5c[t] + FS],
                start=(i == 0), stop=(i == 14),
            )
        for i in range(5):
            tap(nc.vector, cA[:], x1p, w_dwp, [2 * i], off5c, i == 0)
            tap(nc.vector, cB[:], x1p, w_dwp, [2 * i + 1], off5c, i == 0)
        nc.vector.tensor_copy(cC[:], psd[:])
        nc.vector.tensor_tensor(cA[:], cA[:], cB[:], OP.add)
        nc.vector.tensor_tensor(cA[:], cA[:], cC[:], OP.add)
        nc.scalar.activation(x2p[:], cA[:], AF.Silu, bias=b_dwp)

        # unpack x2p into x12 rows 32:64 (image rows 2..21)
        for g in range(4):
            (nc.sync, nc.scalar, nc.gpsimd, nc.gpsimd)[g].dma_start(
                x123[32:64, 2 + 5 * g : 7 + 5 * g, :],
                x2p[32 * g : 32 * g + 32, :].rearrange(
                    "p (r c) -> p r c", c=XW
                )[:, 0:5, 0:W],
            )

        # ---- comp px: 1x1 conv 64->64 (+ SiLU)
        for r0, nr in ((0, 8), (8, 8), (16, 4)):
            ps = psF.tile([64, 512], F32, tag="convps")
            nc.tensor.matmul(
                ps[:, : nr * W], w_px,
                x12[0:64, (2 + r0) * W : (2 + r0 + nr) * W],
                start=True, stop=True,
            )
            nc.scalar.activation(
                enc_in[0:64, r0 * W : (r0 + nr) * W], ps[:, : nr * W],
                AF.Silu, bias=b_px,
            )

        # ---- enc cv1: 1x1 conv 64->50 (+ SiLU, mask row rides K=65) and
        # interleaved e3p/e5p packing (quarter q needs e1c rows 4q:4q+8)
        epack_engs = iter((nc.sync, nc.scalar, nc.gpsimd, nc.scalar,
                           nc.sync, nc.gpsimd, nc.gpsimd, nc.scalar))
        for ci, (r0, nr) in enumerate(((0, 8), (8, 8), (16, 4))):
            ps = psF.tile([50, 512], F32, tag="convps")
            nc.tensor.matmul(
                ps[:, : nr * W], w_ecv1,
                enc_in[0:64, r0 * W : (r0 + nr) * W],
                start=True, stop=False,
            )
            nc.tensor.matmul(
                ps[:, : nr * W], eones,
                mrowem[0:1, XR * W + r0 * W : XR * W + (r0 + nr) * W],
                start=False, stop=True,
            )
            nc.scalar.activation(
                e1c[0:50, r0 * W : (r0 + nr) * W], ps[:, : nr * W],
                AF.Silu, bias=b_ecv1,
            )
            qs = {0: (0,), 1: (1, 2), 2: (3,)}[ci]
            for q in qs:
                next(epack_engs).dma_start(
                    e3p3[32 * q : 32 * q + 25, 0:8, 2 : 2 + W],
                    e1c3[0:25, 4 * q : 4 * q + 8, :],
                )
                next(epack_engs).dma_start(
                    e5p3[32 * q : 32 * q + 25, 0:8, 2 : 2 + W],
                    e1c3[25:50, 4 * q : 4 * q + 8, :],
                )

        # ---- enc dw: dw5 25 taps + dw3 9 taps, split DVE/Pool
        off5 = [(t // 5) * XW + (t % 5) for t in range(25)]
        off3 = [((t // 3) + 1) * XW + (t % 3) + 1 for t in range(9)]
        pse = psF.tile([128, FS2], F32, tag="dwps")
        for i in range(15):
            t = 10 + i
            nc.tensor.matmul(
                pse[:], wdiag[:, 128 * (15 + i) : 128 * (16 + i)],
                e5p[:, off5[t] : off5[t] + FS2],
                start=(i == 0), stop=(i == 14),
            )
        for i in range(5):
            tap(nc.vector, eA[:], e5p, w_e5, [2 * i], off5, i == 0)
            tap(nc.vector, eB[:], e5p, w_e5, [2 * i + 1], off5, i == 0)
        for i in range(5):
            tap(nc.vector, eF[:], e3p, w_e3, [i], off3, i == 0)
            if i < 4:
                tap(nc.vector, eG[:], e3p, w_e3, [5 + i], off3, i == 0)
        nc.vector.tensor_copy(eC[:], pse[:])
        nc.vector.tensor_tensor(eA[:], eA[:], eB[:], OP.add)
        nc.vector.tensor_tensor(eA[:], eA[:], eC[:], OP.add)
        nc.vector.tensor_tensor(eF[:], eF[:], eG[:], OP.add)
        nc.scalar.activation(e5sil[:], eA[:], AF.Silu, bias=b_e5)
        nc.scalar.activation(e3sil[:], eF[:], AF.Silu, bias=b_e3)

        # ---- assemble enc_cat [100, 16W]: e1c interior + enc dw outs
        nc.sync.dma_start(enc_cat[0:50, :], e1c[0:50, 2 * W : 18 * W])
        for q in range(4):
            nc.gpsimd.dma_start(
                enc_cat3[50:75, 4 * q : 4 * q + 4, :],
                e3sil[32 * q : 32 * q + 25, :].rearrange(
                    "p (r c) -> p r c", c=XW
                )[:, 0:4, 0:W],
            )
            (nc.sync if q % 2 == 0 else nc.scalar).dma_start(
                enc_cat3[75:100, 4 * q : 4 * q + 4, :],
                e5sil[32 * q : 32 * q + 25, :].rearrange(
                    "p (r c) -> p r c", c=XW
                )[:, 0:4, 0:W],
            )

        # ---- enc px transposed: WT[sk, pix], 2 chunks of 512
        for hc in range(2):
            ps = psF.tile([100, 512], F32, tag="convps")
            nc.tensor.matmul(
                ps[:], w_epx100, enc_cat[0:100, 512 * hc : 512 * (hc + 1)],
                start=True, stop=True,
            )
            nc.scalar.activation(
                wts[:, 512 * hc : 512 * (hc + 1)], ps[:], AF.Silu, bias=b_epx
            )

        # ---- softmax over 25 taps (transposed); single exp op so the
        # Act queue never alternates Silu/Exp tables
        nc.scalar.activation(ev[:], wts[:], AF.Exp)
        for hc in range(2):
            sl = slice(512 * hc, 512 * (hc + 1))
            ssum = psS.tile([4, 512], F32, tag="ssum")
            nc.tensor.matmul(ssum[:], esel, ev[:, sl], start=True, stop=True)
            with nc.allow_low_precision(reason="bf16 softmax denominators"):
                nc.vector.reciprocal(rv[:, sl], ssum[:])
            rrep = psS.tile([100, 512], F32, tag="rrep")
            nc.tensor.matmul(rrep[:], esel4t, rv[:, sl], start=True, stop=True)
            nc.vector.tensor_tensor(wnorm[:, sl], ev[:, sl], rrep[:], OP.mult)

        psS_cm.__exit__(None, None, None)
        psF_cm.__exit__(None, None, None)
        psB = ctx.enter_context(tc.tile_pool(name="psB", bufs=2, space="PSUM"))
        psC = ctx.enter_context(tc.tile_pool(name="psC", bufs=2, space="PSUM"))
        psO = ctx.enter_context(tc.tile_pool(name="psO", bufs=2, space="PSUM"))
        psW = ctx.enter_context(tc.tile_pool(name="psW", bufs=2, space="PSUM"))

        # ---- backend t-loop: replicate+transpose matmul -> scatter ->
        # PE transpose -> big matmul -> bf16 out
        for t in range(8):
            psw = psW.tile([128, 100], BF16, tag="wcatps")
            nc.tensor.transpose(
                psw[:], wnorm[:, 128 * t : 128 * t + 128], i100,
            )
            wcat = spool.tile([128, 100], BF16, tag="wcat")
            if t % 2 == 0:
                nc.scalar.copy(wcat[:], psw[:])
            else:
                nc.vector.tensor_copy(wcat[:], psw[:])
            psb = psB.tile([128, 400], F32, tag="dallps")
            for jb in range(4):
                nc.tensor.matmul(
                    psb[:, 100 * jb : 100 * jb + 100],
                    repl[:, 128 * jb : 128 * jb + 128], wcat[:],
                    start=True, stop=True,
                )
            dall = spool.tile([128, 400], BF16, tag="dall")
            if t % 2 == 0:
                nc.vector.tensor_copy(dall[:], psb[:])
            else:
                nc.scalar.copy(dall[:], psb[:])

            b4t = spool.tile([128, 480], BF16, tag="b4t")
            nc.gpsimd.local_scatter(
                b4t[:], dall[:], sidx[:],
                channels=128, num_elems=480, num_idxs=400,
            )

            psc = psC.tile([120, 512], BF16, tag="b4ps")
            for jb in range(4):
                nc.tensor.transpose(
                    psc[:, 128 * jb : 128 * jb + 128],
                    b4t[:, 120 * jb : 120 * jb + 120],
                    ident,
                )
            b4 = spool.tile([120, 512], BF16, tag="b4")
            if t % 2 == 0:
                nc.scalar.copy(b4[:], psc[:])
            else:
                nc.vector.tensor_copy(b4[:], psc[:])

            pso = psO.tile([128, 512], F32, tag="outps")
            for jb in range(4):
                nc.tensor.matmul(
                    pso[:, 128 * jb : 128 * jb + 128],
                    xt[:, 512 * t + 128 * jb : 512 * t + 128 * jb + 128],
                    b4[:, 128 * jb : 128 * jb + 128],
                    start=True, stop=True,
                )
            stg = spool.tile([128, 512], BF16, tag="ostage")
            if t % 2 == 0:
                nc.vector.tensor_copy(stg[:], pso[:])
            else:
                nc.scalar.copy(stg[:], pso[:])
            (nc.sync if t % 2 == 0 else nc.scalar).dma_start(
                out_d[:, 512 * t : 512 * (t + 1)], stg[:]
            )

    nc.compile()
    return nc


_NC_CACHE = None


def _get_nc():
    global _NC_CACHE
    if _NC_CACHE is None:
        _NC_CACHE = build_kernel()
    return _NC_CACHE


def kernel(**inputs) -> np.ndarray:
    X = np.asarray(inputs["X"], np.float32)
    consts = _host_consts(
        {k: np.asarray(v, np.float32) for k, v in inputs.items() if k != "X"}
    )
    in_maps = []
    for core in range(NCORES):
        xs, mrowem, xt = _host_shard(X, core)
        m = dict(consts)
        m["x"] = xs
        m["mrowem"] = mrowem
        m["xt"] = xt
        in_maps.append(m)

    nc = _get_nc()
    res = run_bass_kernel_spmd(nc, in_maps, core_ids=list(range(NCORES)))
    out = np.zeros((2, C, 128, 128), np.float32)
    for core in range(NCORES):
        b, ri = divmod(core, 4)
        # stg layout per t: [c, (jb, rho, j32)] -> rows 4t+rho, cols 32jb+j32
        o = res.results[core]["out"].astype(np.float32)
        o = o.reshape(C, 8, 4, 4, 32).transpose(0, 1, 3, 2, 4).reshape(C, 32, 128)
        out[b, :, 32 * ri : 32 * ri + 32, :] = o
    return out


if __name__ == "__main__":
    print("smoke build only")
    build_kernel()
    print("build ok")


# revision 6
# speedup vs baseline: 1.0863x; 1.0863x over previous
"""CARAFE + MSGConv Trainium2 kernel v2 (8 NeuronCores, spatial x batch sharding).

out[c, i, j] = sum_{p,q} W[5p+q, i, j] * Xpad[c, i//2 + p - 2, j//2 + q - 2]

Per core: one batch element (core//4), a 16-source-row block (core%4).
v2 changes vs baseline:
 - depthwise tap MACs split across DVE and GPSIMD (Pool) engines
 - enc dw repacked into two [128, 272]-wide slabs, 32-aligned row quarters
   (dw3 runs only its 9 taps)
 - dw outputs consumed in packed form by the next 1x1 conv via K-split
   accumulating matmuls at legal base partitions {0,32,64}; only the
   4th group needs a fix-up DMA to a base-32/64 scratch block
 - encoder px computed transposed ([100 sk, 1024 pix]); softmax via
   matmul column-sums + reciprocal + replicate-matmul + elementwise mult
 - backend: transpose+replicate fused into one matmul per (t,jb) using a
   broadcast access pattern on lhsT; batched PSUM->SBUF copies; bf16 out
"""

import sys

sys.path.insert(0, "/opt/trn_rl_repo")

from contextlib import ExitStack

import ml_dtypes
import numpy as np

import concourse.bass as bass
import concourse.tile as tile
from concourse import bacc, library_config, mybir
from concourse.ap import AP
from concourse.bass_utils import run_bass_kernel_spmd

BF16 = mybir.dt.bfloat16
F32 = mybir.dt.float32
I16 = mybir.dt.int16
AF = mybir.ActivationFunctionType
OP = mybir.AluOpType
nbf = ml_dtypes.bfloat16

C = 128
H = W = 64
NCORES = 8
XR = 24          # X shard rows (16 + 4 halo each side)
XW = 68          # padded slab pitch
NEG = -30.0      # additive pre-activation mask; SiLU(-30) ~= -2.8e-12

# packa column layout (bf16 [128, 904])
PA_CV1 = 0       # w_cv1 [128, 32]
PA_PXLO = 32     # w_px rows 0:32  [32, 64]
PA_PXHI = 96     # w_px rows 32:64, replicated at bases 0/32/64 [96, 64]
PA_ECV1 = 160    # w_ecv1 [65, 50]
PA_EPXA = 210    # w_epx rows 0:50 [50, 100]
PA_EPXB = 310    # w_epx rows 50:75, replicated at bases 0/32/64 [89, 100]
PA_EPXC = 410    # w_epx rows 75:100, replicated at bases 0/32/64 [89, 100]
PA_ID = 510      # identity [128, 128]
PA_ESEL = 638    # esel [100, 4]
PA_ESELT = 642   # esel4T [4, 100]
PA_ONES = 742    # ones [1, 32]
PA_EONES = 774   # ones [1, 50]
PA_WIDTH = 832

# packb column layout (f32 [128, 66])
PB_WDWP = 0      # comp dw taps [128, 25]
PB_WE3 = 25      # enc dw3 taps [128, 9]  (32-aligned quarters)
PB_WE5 = 34      # enc dw5 taps [128, 25]
PB_BDWP = 59     # comp dw bias [128, 1]
PB_BE3 = 60      # enc dw3 bias [128, 1]
PB_BE5 = 61      # enc dw5 bias [128, 1]
PB_BCV1 = 62     # [32, 1]
PB_BPX = 63      # [64, 1]
PB_BECV1 = 64    # [50, 1]
PB_BEPX = 65     # [100, 1]
PB_WIDTH = 66


# ======================================================================
# host-side parameter prep
# ======================================================================

def _fold_1x1(w, s):
    return (w[:, :, 0, 0] * s[:, None]).T.copy()


def _dw_taps(w, s, k):
    ch = w.shape[0]
    out = np.zeros((ch, 25), np.float32)
    off = (5 - k) // 2
    for ty in range(k):
        for tx in range(k):
            out[:, 5 * (ty + off) + (tx + off)] = w[:, 0, ty, tx] * s
    return out


def _pad32(a):
    """[25ch x 4q] tap table -> [128] with 32-aligned quarters."""
    out = np.zeros((128,) + a.shape[1:], a.dtype)
    for q in range(4):
        out[32 * q:32 * q + 25] = a[25 * q:25 * q + 25]
    return out


def _host_consts(inputs):
    d = {}
    w_cv1 = _fold_1x1(inputs["comp_cv1_w"], inputs["comp_cv1_s"])     # [128,32]
    w3 = _dw_taps(inputs["comp_dw3_w"], inputs["comp_dw3_s"], 3)
    w5 = _dw_taps(inputs["comp_dw5_w"], inputs["comp_dw5_s"], 5)
    w_dwp = np.tile(np.concatenate([w3, w5], 0), (4, 1))              # [128,25]
    b_dwp = np.tile(
        np.concatenate([inputs["comp_dw3_b"], inputs["comp_dw5_b"]]), 4
    ).reshape(128, 1)
    w_px = _fold_1x1(inputs["comp_px_w"], inputs["comp_px_s"])        # [64,64]
    w_ecv1 = _fold_1x1(inputs["enc_cv1_w"], inputs["enc_cv1_s"])      # [64,50]
    # enc dw taps in [25ch x 4 row-quarters] -> padded to 32-aligned
    e3 = inputs["enc_dw3_w"][:, 0] * inputs["enc_dw3_s"][:, None, None]
    e5 = inputs["enc_dw5_w"][:, 0] * inputs["enc_dw5_s"][:, None, None]
    w_e3 = _pad32(np.tile(e3.reshape(25, 9), (4, 1)))                 # [128,9]
    w_e5 = _pad32(np.tile(e5.reshape(25, 25), (4, 1)))                # [128,25]
    b_e3 = _pad32(np.tile(inputs["enc_dw3_b"], 4).reshape(100, 1))
    b_e5 = _pad32(np.tile(inputs["enc_dw5_b"], 4).reshape(100, 1))
    w_epx = _fold_1x1(inputs["enc_px_w"], inputs["enc_px_s"])         # [100,100]

    pa = np.zeros((128, PA_WIDTH), np.float32)
    pa[0:128, PA_CV1:PA_CV1 + 32] = w_cv1
    pa[0:64, PA_PXLO:PA_PXLO + 64] = w_px
    pa[0:64, PA_ECV1:PA_ECV1 + 50] = w_ecv1
    pa[0:100, PA_EPXA:PA_EPXA + 100] = w_epx
    # K-split weights replicated at bases 0/32/64 so lhsT base partition
    # matches the packed rhs slab's base partition
    for b0 in (0, 32, 64):
        pa[b0:b0 + 32, PA_PXHI:PA_PXHI + 64] = w_px[32:64]
        pa[b0:b0 + 25, PA_EPXB:PA_EPXB + 100] = w_epx[50:75]
        pa[b0:b0 + 25, PA_EPXC:PA_EPXC + 100] = w_epx[75:100]
    pa[0:128, PA_ID:PA_ID + 128] = np.eye(128)
    for s in range(4):
        for k in range(25):
            pa[4 * k + s, PA_ESEL + s] = 1.0                          # esel
            pa[s, PA_ESELT + 4 * k + s] = 1.0                         # esel4T
    pa[0, PA_ONES:PA_ONES + 32] = 1.0
    pa[0, PA_EONES:PA_EONES + 50] = 1.0
    d["packa"] = pa.astype(nbf)

    pb = np.zeros((128, PB_WIDTH), np.float32)
    pb[:, PB_WDWP:PB_WDWP + 25] = w_dwp
    pb[:, PB_WE3:PB_WE3 + 9] = w_e3
    pb[:, PB_WE5:PB_WE5 + 25] = w_e5
    pb[:, PB_BDWP:PB_BDWP + 1] = b_dwp
    pb[:, PB_BE3:PB_BE3 + 1] = b_e3
    pb[:, PB_BE5:PB_BE5 + 1] = b_e5
    pb[0:32, PB_BCV1:PB_BCV1 + 1] = inputs["comp_cv1_b"].reshape(32, 1)
    pb[0:64, PB_BPX:PB_BPX + 1] = inputs["comp_px_b"].reshape(64, 1)
    pb[0:50, PB_BECV1:PB_BECV1 + 1] = inputs["enc_cv1_b"].reshape(50, 1)
    pb[0:100, PB_BEPX:PB_BEPX + 1] = inputs["enc_px_b"].reshape(100, 1)
    d["packb"] = pb

    # wdiag [128, 30*128]: diagonal tap weights for PE-side dw taps
    # cols 0..14  -> comp taps 10..24 (w_dwp)
    # cols 15..29 -> enc dw5 taps 10..24 (w_e5)
    wd = np.zeros((128, 30 * 128), np.float32)
    for i in range(15):
        np.fill_diagonal(wd[:, 128 * i:128 * (i + 1)], w_dwp[:, 10 + i])
        np.fill_diagonal(wd[:, 128 * (15 + i):128 * (16 + i)], w_e5[:, 10 + i])
    d["wdiag"] = wd.astype(nbf)

    # repl [128, 4*128]: lhsT for the W row-replication matmul
    rp = np.zeros((128, 512), np.float32)
    for jb in range(4):
        for n in range(128):
            rho, j = divmod(n, 32)
            yl, xl = rho // 2, j // 2
            rp[64 * yl + 16 * jb + xl, 128 * jb + n] = 1.0
    d["repl"] = rp.astype(nbf)

    # sidx [128, 4*100] int16; horizontal out-of-image taps dropped (-1).
    si = np.full((128, 400), -1, np.int16)
    for n in range(128):
        rho, j = divmod(n, 32)
        yl, dy = divmod(rho, 2)
        xl, dx = divmod(j, 2)
        sn = 2 * dy + dx
        for jb in range(4):
            for cp in range(100):
                k, sc = divmod(cp, 4)
                if sc != sn:
                    continue
                p, q = divmod(k, 5)
                if not (0 <= 16 * jb + xl + q - 2 < 64):
                    continue
                si[n, 100 * jb + cp] = 120 * jb + 20 * (yl + p) + (xl + q)
    d["sidx"] = si
    return d


def _host_shard(X, core):
    b, ri = divmod(core, 4)
    r0 = 16 * ri - 4
    xs = np.zeros((C, XR, W), np.float32)
    lo, hi = max(0, r0), min(H, r0 + XR)
    xs[:, lo - r0 : hi - r0, :] = X[b, :, lo:hi, :]
    mrow = np.zeros((1, XR, W), np.float32)
    for r in range(XR):
        if not (0 <= r0 + r < H):
            mrow[0, r, :] = NEG
    emask = np.zeros((1, 20, W), np.float32)
    for r in range(20):
        if not (0 <= (16 * ri - 2) + r < H):
            emask[0, r, :] = NEG
    xsb = xs.astype(nbf)
    # pre-transposed X slabs, one [120, 128] per block (column-padded)
    xsp = np.zeros((C, XR, XW), nbf)
    xsp[:, :, 2 : 2 + W] = xsb
    xt = np.zeros((120, 32 * 128), nbf)
    for B in range(32):
        t, jb = divmod(B, 4)
        slab = xsp[:, 2 * t + 2 : 2 * t + 8, 16 * jb : 16 * jb + 20]
        xt[:, 128 * B : 128 * B + 128] = slab.reshape(C, 120).T
    mrowem = np.zeros((1, 2 * XR * W), np.float32)
    mrowem[0, 0 : XR * W] = mrow.reshape(XR * W)
    mrowem[0, XR * W : XR * W + 20 * W] = emask.reshape(20 * W)
    return (
        xsb.reshape(C, XR * W),
        mrowem.astype(nbf),
        xt,
    )


# ======================================================================
# device kernel
# ======================================================================

def build_kernel():
    nc = bacc.Bacc(
        "TRN2",
        target_bir_lowering=False,
        debug=False,
        enable_asserts=False,
        num_devices=NCORES,
    )

    def din(name, shape, dt):
        return nc.dram_tensor(name, list(shape), dt, kind="ExternalInput").ap()

    x_d = din("x", (128, XR * W), BF16)
    xt_d = din("xt", (120, 32 * 128), BF16)
    mrowem_d = din("mrowem", (1, 2 * XR * W), BF16)
    packa_d = din("packa", (128, PA_WIDTH), BF16)
    packb_d = din("packb", (128, PB_WIDTH), F32)
    repl_d = din("repl", (128, 512), BF16)
    wdiag_d = din("wdiag", (128, 30 * 128), BF16)
    sidx_d = din("sidx", (128, 400), I16)
    out_d = nc.dram_tensor("out", [128, 32 * 128], BF16, kind="ExternalOutput").ap()

    FS = 5 * XW                    # comp tap width 340
    FS2 = 4 * XW                   # enc tap width 272

    with tile.TileContext(nc) as tc, ExitStack() as ctx:
        cpool = ctx.enter_context(tc.tile_pool(name="consts", bufs=1))
        work = ctx.enter_context(tc.tile_pool(name="work", bufs=1))
        psF_cm = tc.tile_pool(name="psF", bufs=2, space="PSUM")
        psF = psF_cm.__enter__()
        psS_cm = tc.tile_pool(name="psS", bufs=1, space="PSUM")
        psS = psS_cm.__enter__()
        spool = ctx.enter_context(tc.tile_pool(name="stage", bufs=3))

        nc.gpsimd.load_library(library_config.local_scatter)

        packa = cpool.tile([128, PA_WIDTH], BF16, tag="packa")
        packb = cpool.tile([128, PB_WIDTH], F32, tag="packb")
        mrowem = cpool.tile([1, 2 * XR * W], BF16, tag="mrowem")
        xb = cpool.tile([128, XR * W], BF16, tag="x")
        xt = cpool.tile([120, 32 * 128], BF16, tag="xt")
        sidx = cpool.tile([128, 400], I16, tag="sidx")
        repl = cpool.tile([128, 512], BF16, tag="repl")
        wdiag = cpool.tile([128, 30 * 128], BF16, tag="wdiag")

        # sync queue: x first, then xt; scalar queue: weights + masks
        nc.sync.dma_start(xb[:], x_d)
        nc.scalar.dma_start(packa[:], packa_d)
        nc.scalar.dma_start(mrowem[:], mrowem_d)
        nc.sync.dma_start(xt[:], xt_d)
        nc.scalar.dma_start(packb[:], packb_d)
        nc.gpsimd.dma_start(sidx[:], sidx_d)
        nc.gpsimd.dma_start(repl[:], repl_d)
        nc.scalar.dma_start(wdiag[:], wdiag_d)

        w_cv1 = packa[0:128, PA_CV1:PA_CV1 + 32]
        w_px = packa[0:64, PA_PXLO:PA_PXLO + 64]
        w_ecv1 = packa[0:64, PA_ECV1:PA_ECV1 + 50]
        w_epx100 = packa[0:100, PA_EPXA:PA_EPXA + 100]
        ident = packa[0:128, PA_ID:PA_ID + 128]
        i100 = packa[0:100, PA_ID:PA_ID + 100]
        esel = packa[0:100, PA_ESEL:PA_ESEL + 4]
        esel4t = packa[0:4, PA_ESELT:PA_ESELT + 100]
        ones1 = packa[0:1, PA_ONES:PA_ONES + 32]
        eones = packa[0:1, PA_EONES:PA_EONES + 50]
        w_dwp = packb[0:128, PB_WDWP:PB_WDWP + 25]
        w_e3 = packb[0:128, PB_WE3:PB_WE3 + 9]
        w_e5 = packb[0:128, PB_WE5:PB_WE5 + 25]
        b_dwp = packb[0:128, PB_BDWP:PB_BDWP + 1]
        b_e3 = packb[0:128, PB_BE3:PB_BE3 + 1]
        b_e5 = packb[0:128, PB_BE5:PB_BE5 + 1]
        b_cv1 = packb[0:32, PB_BCV1:PB_BCV1 + 1]
        b_px = packb[0:64, PB_BPX:PB_BPX + 1]
        b_ecv1 = packb[0:50, PB_BECV1:PB_BECV1 + 1]
        b_epx = packb[0:100, PB_BEPX:PB_BEPX + 1]

        def pxhi(b0):
            return packa[b0:b0 + 32, PA_PXHI:PA_PXHI + 64]

        def epxb(b0):
            return packa[b0:b0 + 25, PA_EPXB:PA_EPXB + 100]

        def epxc(b0):
            return packa[b0:b0 + 25, PA_EPXC:PA_EPXC + 100]

        # ---- working tensors
        x12 = work.tile([64, XR * W], BF16)          # x1 (0:32) + x2 (32:64)
        enc_cat = work.tile([100, 16 * W], BF16)     # e1c + enc dw outs
        x1p = work.tile([128, 9 * XW + 8], BF16)     # packed x1 (68-pitch)
        x2p = work.tile([128, FS], BF16)             # comp dw out (packed)
        enc_in = work.tile([64, 20 * W], BF16)       # px out
        e1c = work.tile([50, 20 * W], BF16)          # enc cv1 out
        e3p = work.tile([128, 8 * XW + 8], BF16)     # enc dw3 slab
        e5p = work.tile([128, 8 * XW + 8], BF16)     # enc dw5 slab
        e3sil = work.tile([128, FS2], BF16)          # enc dw3 out (packed)
        e5sil = work.tile([128, FS2], BF16)          # enc dw5 out (packed)
        wts = work.tile([100, 1024], BF16)           # silu(enc px), transposed
        ev = work.tile([100, 1024], BF16)            # exp(wts)
        rv = work.tile([4, 1024], BF16)              # 1/colsum per subpixel
        wnorm = work.tile([100, 1024], BF16)         # softmaxed weights [sk,pix]
        # wcat_t: per-t transposed weights [pix-in-rowpair, sk]
        cA = work.tile([128, FS], BF16)
        cB = work.tile([128, FS], BF16)
        cC = work.tile([128, FS], BF16)
        cD = work.tile([128, FS], BF16)
        eA = work.tile([128, FS2], BF16)
        eB = work.tile([128, FS2], BF16)
        eC = work.tile([128, FS2], BF16)
        eF = work.tile([128, FS2], BF16)
        eG = work.tile([128, FS2], BF16)

        # warmup: trigger local_scatter ucode load early
        warm = work.tile([16, 16], BF16)
        nc.gpsimd.local_scatter(
            warm[:], packa[0:16, 0:2], sidx[:][0:16, 0:2],
            channels=16, num_elems=16, num_idxs=2,
        )
        # warmup: force silu act-table loads while inputs stream in
        wsin = work.tile([1, 2], BF16)
        wsout = work.tile([1, 2], BF16)
        nc.vector.memset(wsin[:], 0.0)
        nc.scalar.activation(wsout[:], wsin[:], AF.Silu)

        x123 = x12[:].rearrange("p (r c) -> p r c", c=W)
        enc_cat3 = enc_cat[:].rearrange("p (r c) -> p r c", c=W)
        e1c3 = e1c[:].rearrange("p (r c) -> p r c", c=W)
        x1p3 = x1p[:, 0 : 9 * XW].rearrange("p (r c) -> p r c", c=XW)
        e3p3 = e3p[:, 0 : 8 * XW].rearrange("p (r c) -> p r c", c=XW)
        e5p3 = e5p[:, 0 : 8 * XW].rearrange("p (r c) -> p r c", c=XW)

        # zero slab pad columns + pad partitions (25:32 per quarter)
        nc.vector.memset(x1p[:, 9 * XW : 9 * XW + 8], 0.0)
        nc.vector.memset(x1p3[:, :, 0:2], 0.0)
        nc.vector.memset(x1p3[:, :, 66:68], 0.0)
        nc.vector.memset(e3p[:], 0.0)
        nc.vector.memset(e5p[:], 0.0)

        # ---- comp cv1 + interleaved x1p packing
        for ch in range(3):
            ps = psF.tile([32, 512], F32, tag="convps")
            nc.tensor.matmul(
                ps[:], w_cv1, xb[:, 512 * ch : 512 * (ch + 1)],
                start=True, stop=False,
            )
            nc.tensor.matmul(
                ps[:], ones1, mrowem[0:1, 512 * ch : 512 * (ch + 1)],
                start=False, stop=True,
            )
            nc.scalar.activation(
                x12[0:32, 512 * ch : 512 * (ch + 1)], ps[:],
                AF.Silu, bias=b_cv1,
            )
            # pack row-groups as soon as their rows exist:
            # g0 needs rows 0:9 (ch0+ch1), g1 rows 5:14, g2 rows 10:19,
            # g3 rows 15:24 (ch2)
            packs = {1: (0, 1), 2: (2, 3)}.get(ch, ())
            for g in packs:
                eng = (nc.gpsimd, nc.gpsimd, nc.sync, nc.sync)[g]
                eng.dma_start(
                    x1p3[32 * g : 32 * g + 32, 0:9, 2 : 2 + W],
                    x123[0:32, 5 * g : 5 * g + 9, :],
                )

        # ---- comp dw: 25 unified 5x5 taps, DVE (15) + Pool (10)
        def tap(eng, acc, slab, wsel, taps, offs, first):
            for i, t in enumerate(taps):
                sv = slab[:, offs[t] : offs[t] + (FS if slab is x1p else FS2)]
                if i == 0 and first:
                    eng.tensor_scalar(acc, sv, wsel[:, t : t + 1], None, OP.mult)
                else:
                    eng.scalar_tensor_tensor(
                        acc, sv, wsel[:, t : t + 1], acc, OP.mult, OP.add
                    )

        off5c = [(t // 5) * XW + (t % 5) for t in range(25)]
        psd = psF.tile([128, FS], F32, tag="dwps")
        for i in range(15):
            t = 10 + i
            nc.tensor.matmul(
                psd[:], wdiag[:, 128 * i : 128 * (i + 1)],
                x1p[:, off5c[t] : off5c[t] + FS],
                start=(i == 0), stop=(i == 14),
            )
        for i in range(5):
            tap(nc.vector, cA[:], x1p, w_dwp, [2 * i], off5c, i == 0)
            tap(nc.vector, cB[:], x1p, w_dwp, [2 * i + 1], off5c, i == 0)
        nc.vector.tensor_copy(cC[:], psd[:])
        nc.vector.tensor_tensor(cA[:], cA[:], cB[:], OP.add)
        nc.vector.tensor_tensor(cA[:], cA[:], cC[:], OP.add)
        nc.scalar.activation(x2p[:], cA[:], AF.Silu, bias=b_dwp)

        # unpack x2p into x12 rows 32:64 (image rows 2..21)
        for g in range(4):
            (nc.sync, nc.sync, nc.gpsimd, nc.gpsimd)[g].dma_start(
                x123[32:64, 2 + 5 * g : 7 + 5 * g, :],
                x2p[32 * g : 32 * g + 32, :].rearrange(
                    "p (r c) -> p r c", c=XW
                )[:, 0:5, 0:W],
            )

        # ---- comp px: 1x1 conv 64->64 (+ SiLU)
        for r0, nr in ((0, 8), (8, 8), (16, 4)):
            ps = psF.tile([64, 512], F32, tag="convps")
            nc.tensor.matmul(
                ps[:, : nr * W], w_px,
                x12[0:64, (2 + r0) * W : (2 + r0 + nr) * W],
                start=True, stop=True,
            )
            nc.scalar.activation(
                enc_in[0:64, r0 * W : (r0 + nr) * W], ps[:, : nr * W],
                AF.Silu, bias=b_px,
            )

        # ---- enc cv1: 1x1 conv 64->50 (+ SiLU, mask row rides K=65) and
        # interleaved e3p/e5p packing (quarter q needs e1c rows 4q:4q+8)
        epack_engs = iter((nc.sync, nc.gpsimd, nc.sync, nc.gpsimd,
                           nc.sync, nc.gpsimd, nc.sync, nc.sync))
        for ci, (r0, nr) in enumerate(((0, 8), (8, 8), (16, 4))):
            ps = psF.tile([50, 512], F32, tag="convps")
            nc.tensor.matmul(
                ps[:, : nr * W], w_ecv1,
                enc_in[0:64, r0 * W : (r0 + nr) * W],
                start=True, stop=False,
            )
            nc.tensor.matmul(
                ps[:, : nr * W], eones,
                mrowem[0:1, XR * W + r0 * W : XR * W + (r0 + nr) * W],
                start=False, stop=True,
            )
            nc.scalar.activation(
                e1c[0:50, r0 * W : (r0 + nr) * W], ps[:, : nr * W],
                AF.Silu, bias=b_ecv1,
            )
            qs = {0: (0,), 1: (1, 2), 2: (3,)}[ci]
            for q in qs:
                next(epack_engs).dma_start(
                    e3p3[32 * q : 32 * q + 25, 0:8, 2 : 2 + W],
                    e1c3[0:25, 4 * q : 4 * q + 8, :],
                )
                next(epack_engs).dma_start(
                    e5p3[32 * q : 32 * q + 25, 0:8, 2 : 2 + W],
                    e1c3[25:50, 4 * q : 4 * q + 8, :],
                )

        # ---- enc dw: dw5 25 taps + dw3 9 taps, split DVE/Pool
        off5 = [(t // 5) * XW + (t % 5) for t in range(25)]
        off3 = [((t // 3) + 1) * XW + (t % 3) + 1 for t in range(9)]
        pse = psF.tile([128, FS2], F32, tag="dwps")
        for i in range(15):
            t = 10 + i
            nc.tensor.matmul(
                pse[:], wdiag[:, 128 * (15 + i) : 128 * (16 + i)],
                e5p[:, off5[t] : off5[t] + FS2],
                start=(i == 0), stop=(i == 14),
            )
        for i in range(5):
            tap(nc.vector, eA[:], e5p, w_e5, [2 * i], off5, i == 0)
            tap(nc.vector, eB[:], e5p, w_e5, [2 * i + 1], off5, i == 0)
        for i in range(5):
            tap(nc.vector, eF[:], e3p, w_e3, [i], off3, i == 0)
            if i < 4:
                tap(nc.vector, eG[:], e3p, w_e3, [5 + i], off3, i == 0)
        nc.vector.tensor_copy(eC[:], pse[:])
        nc.vector.tensor_tensor(eA[:], eA[:], eB[:], OP.add)
        nc.vector.tensor_tensor(eA[:], eA[:], eC[:], OP.add)
        nc.vector.tensor_tensor(eF[:], eF[:], eG[:], OP.add)
        nc.scalar.activation(e5sil[:], eA[:], AF.Silu, bias=b_e5)
        nc.scalar.activation(e3sil[:], eF[:], AF.Silu, bias=b_e3)

        # ---- assemble enc_cat [100, 16W]: e1c interior + enc dw outs
        nc.sync.dma_start(enc_cat[0:50, :], e1c[0:50, 2 * W : 18 * W])
        for q in range(4):
            (nc.gpsimd if q < 2 else nc.sync).dma_start(
                enc_cat3[50:75, 4 * q : 4 * q + 4, :],
                e3sil[32 * q : 32 * q + 25, :].rearrange(
                    "p (r c) -> p r c", c=XW
                )[:, 0:4, 0:W],
            )
            nc.sync.dma_start(
                enc_cat3[75:100, 4 * q : 4 * q + 4, :],
                e5sil[32 * q : 32 * q + 25, :].rearrange(
                    "p (r c) -> p r c", c=XW
                )[:, 0:4, 0:W],
            )

        # ---- enc px transposed: WT[sk, pix], 2 chunks of 512
        for hc in range(2):
            ps = psF.tile([100, 512], F32, tag="convps")
            nc.tensor.matmul(
                ps[:], w_epx100, enc_cat[0:100, 512 * hc : 512 * (hc + 1)],
                start=True, stop=True,
            )
            nc.scalar.activation(
                wts[:, 512 * hc : 512 * (hc + 1)], ps[:], AF.Silu, bias=b_epx
            )

        # ---- softmax over 25 taps (transposed); single exp op so the
        # Act queue never alternates Silu/Exp tables
        nc.scalar.activation(ev[:], wts[:], AF.Exp)
        for hc in range(2):
            sl = slice(512 * hc, 512 * (hc + 1))
            ssum = psS.tile([4, 512], F32, tag="ssum")
            nc.tensor.matmul(ssum[:], esel, ev[:, sl], start=True, stop=True)
            with nc.allow_low_precision(reason="bf16 softmax denominators"):
                nc.vector.reciprocal(rv[:, sl], ssum[:])
            rrep = psS.tile([100, 512], F32, tag="rrep")
            nc.tensor.matmul(rrep[:], esel4t, rv[:, sl], start=True, stop=True)
            nc.vector.tensor_tensor(wnorm[:, sl], ev[:, sl], rrep[:], OP.mult)

        psS_cm.__exit__(None, None, None)
        psF_cm.__exit__(None, None, None)
        psB = ctx.enter_context(tc.tile_pool(name="psB", bufs=2, space="PSUM"))
        psC = ctx.enter_context(tc.tile_pool(name="psC", bufs=2, space="PSUM"))
        psO = ctx.enter_context(tc.tile_pool(name="psO", bufs=2, space="PSUM"))
        psW = ctx.enter_context(tc.tile_pool(name="psW", bufs=2, space="PSUM"))

        # ---- backend t-loop: replicate+transpose matmul -> scatter ->
        # PE transpose -> big matmul -> bf16 out
        for t in range(8):
            psw = psW.tile([128, 100], BF16, tag="wcatps")
            nc.tensor.transpose(
                psw[:], wnorm[:, 128 * t : 128 * t + 128], i100,
            )
            wcat = spool.tile([128, 100], BF16, tag="wcat")
            if t % 2 == 0:
                nc.scalar.copy(wcat[:], psw[:])
            else:
                nc.vector.tensor_copy(wcat[:], psw[:])
            psb = psB.tile([128, 400], F32, tag="dallps")
            for jb in range(4):
                nc.tensor.matmul(
                    psb[:, 100 * jb : 100 * jb + 100],
                    repl[:, 128 * jb : 128 * jb + 128], wcat[:],
                    start=True, stop=True,
                )
            dall = spool.tile([128, 400], BF16, tag="dall")
            if t % 2 == 0:
                nc.vector.tensor_copy(dall[:], psb[:])
            else:
                nc.scalar.copy(dall[:], psb[:])

            b4t = spool.tile([128, 480], BF16, tag="b4t")
            nc.gpsimd.local_scatter(
                b4t[:], dall[:], sidx[:],
                channels=128, num_elems=480, num_idxs=400,
            )

            psc = psC.tile([120, 512], BF16, tag="b4ps")
            for jb in range(4):
                nc.tensor.transpose(
                    psc[:, 128 * jb : 128 * jb + 128],
                    b4t[:, 120 * jb : 120 * jb + 120],
                    ident,
                )
            b4 = spool.tile([120, 512], BF16, tag="b4")
            if t % 2 == 0:
                nc.scalar.copy(b4[:], psc[:])
            else:
                nc.vector.tensor_copy(b4[:], psc[:])

            pso = psO.tile([128, 512], F32, tag="outps")
            for jb in range(4):
                nc.tensor.matmul(
                    pso[:, 128 * jb : 128 * jb + 128],
                    xt[:, 512 * t + 128 * jb : 512 * t + 128 * jb + 128],
                    b4[:, 128 * jb : 128 * jb + 128],
                    start=True, stop=True,
                )
            stg = spool.tile([128, 512], BF16, tag="ostage")
            if t % 2 == 0:
                nc.vector.tensor_copy(stg[:], pso[:])
            else:
                nc.scalar.copy(stg[:], pso[:])
            nc.sync.dma_start(
                out_d[:, 512 * t : 512 * (t + 1)], stg[:]
            )

    nc.compile()
    return nc


_NC_CACHE = None


def _get_nc():
    global _NC_CACHE
    if _NC_CACHE is None:
        _NC_CACHE = build_kernel()
    return _NC_CACHE


def kernel(**inputs) -> np.ndarray:
    X = np.asarray(inputs["X"], np.float32)
    consts = _host_consts(
        {k: np.asarray(v, np.float32) for k, v in inputs.items() if k != "X"}
    )
    in_maps = []
    for core in range(NCORES):
        xs, mrowem, xt = _host_shard(X, core)
        m = dict(consts)
        m["x"] = xs
        m["mrowem"] = mrowem
        m["xt"] = xt
        in_maps.append(m)

    nc = _get_nc()
    res = run_bass_kernel_spmd(nc, in_maps, core_ids=list(range(NCORES)))
    out = np.zeros((2, C, 128, 128), np.float32)
    for core in range(NCORES):
        b, ri = divmod(core, 4)
        # stg layout per t: [c, (jb, rho, j32)] -> rows 4t+rho, cols 32jb+j32
        o = res.results[core]["out"].astype(np.float32)
        o = o.reshape(C, 8, 4, 4, 32).transpose(0, 1, 3, 2, 4).reshape(C, 32, 128)
        out[b, :, 32 * ri : 32 * ri + 32, :] = o
    return out


if __name__ == "__main__":
    print("smoke build only")
    build_kernel()
    print("build ok")


# revision 7
# speedup vs baseline: 1.0951x; 1.0081x over previous
"""CARAFE + MSGConv Trainium2 kernel v2 (8 NeuronCores, spatial x batch sharding).

out[c, i, j] = sum_{p,q} W[5p+q, i, j] * Xpad[c, i//2 + p - 2, j//2 + q - 2]

Per core: one batch element (core//4), a 16-source-row block (core%4).
v2 changes vs baseline:
 - depthwise tap MACs split across DVE and GPSIMD (Pool) engines
 - enc dw repacked into two [128, 272]-wide slabs, 32-aligned row quarters
   (dw3 runs only its 9 taps)
 - dw outputs consumed in packed form by the next 1x1 conv via K-split
   accumulating matmuls at legal base partitions {0,32,64}; only the
   4th group needs a fix-up DMA to a base-32/64 scratch block
 - encoder px computed transposed ([100 sk, 1024 pix]); softmax via
   matmul column-sums + reciprocal + replicate-matmul + elementwise mult
 - backend: transpose+replicate fused into one matmul per (t,jb) using a
   broadcast access pattern on lhsT; batched PSUM->SBUF copies; bf16 out
"""

import sys

sys.path.insert(0, "/opt/trn_rl_repo")

from contextlib import ExitStack

import ml_dtypes
import numpy as np

import concourse.bass as bass
import concourse.tile as tile
from concourse import bacc, library_config, mybir
from concourse.ap import AP
from concourse.bass_utils import run_bass_kernel_spmd

BF16 = mybir.dt.bfloat16
F32 = mybir.dt.float32
I16 = mybir.dt.int16
AF = mybir.ActivationFunctionType
OP = mybir.AluOpType
nbf = ml_dtypes.bfloat16

C = 128
H = W = 64
NCORES = 8
XR = 24          # X shard rows (16 + 4 halo each side)
XW = 68          # padded slab pitch
NEG = -30.0      # additive pre-activation mask; SiLU(-30) ~= -2.8e-12

# packa column layout (bf16 [128, 904])
PA_CV1 = 0       # w_cv1 [128, 32]
PA_PXLO = 32     # w_px rows 0:32  [32, 64]
PA_PXHI = 96     # w_px rows 32:64, replicated at bases 0/32/64 [96, 64]
PA_ECV1 = 160    # w_ecv1 [65, 50]
PA_EPXA = 210    # w_epx rows 0:50 [50, 100]
PA_EPXB = 310    # w_epx rows 50:75, replicated at bases 0/32/64 [89, 100]
PA_EPXC = 410    # w_epx rows 75:100, replicated at bases 0/32/64 [89, 100]
PA_ID = 510      # identity [128, 128]
PA_ESEL = 638    # esel [100, 4]
PA_ESELT = 642   # esel4T [4, 100]
PA_ONES = 742    # ones [1, 32]
PA_EONES = 774   # ones [1, 50]
PA_WIDTH = 832

# packb column layout (f32 [128, 66])
PB_WDWP = 0      # comp dw taps [128, 25]
PB_WE3 = 25      # enc dw3 taps [128, 9]  (32-aligned quarters)
PB_WE5 = 34      # enc dw5 taps [128, 25]
PB_BDWP = 59     # comp dw bias [128, 1]
PB_BE3 = 60      # enc dw3 bias [128, 1]
PB_BE5 = 61      # enc dw5 bias [128, 1]
PB_BCV1 = 62     # [32, 1]
PB_BPX = 63      # [64, 1]
PB_BECV1 = 64    # [50, 1]
PB_BEPX = 65     # [100, 1]
PB_WIDTH = 66


# ======================================================================
# host-side parameter prep
# ======================================================================

def _fold_1x1(w, s):
    return (w[:, :, 0, 0] * s[:, None]).T.copy()


def _dw_taps(w, s, k):
    ch = w.shape[0]
    out = np.zeros((ch, 25), np.float32)
    off = (5 - k) // 2
    for ty in range(k):
        for tx in range(k):
            out[:, 5 * (ty + off) + (tx + off)] = w[:, 0, ty, tx] * s
    return out


def _pad32(a):
    """[25ch x 4q] tap table -> [128] with 32-aligned quarters."""
    out = np.zeros((128,) + a.shape[1:], a.dtype)
    for q in range(4):
        out[32 * q:32 * q + 25] = a[25 * q:25 * q + 25]
    return out


def _host_consts(inputs):
    d = {}
    w_cv1 = _fold_1x1(inputs["comp_cv1_w"], inputs["comp_cv1_s"])     # [128,32]
    w3 = _dw_taps(inputs["comp_dw3_w"], inputs["comp_dw3_s"], 3)
    w5 = _dw_taps(inputs["comp_dw5_w"], inputs["comp_dw5_s"], 5)
    w_dwp = np.tile(np.concatenate([w3, w5], 0), (4, 1))              # [128,25]
    b_dwp = np.tile(
        np.concatenate([inputs["comp_dw3_b"], inputs["comp_dw5_b"]]), 4
    ).reshape(128, 1)
    w_px = _fold_1x1(inputs["comp_px_w"], inputs["comp_px_s"])        # [64,64]
    w_ecv1 = _fold_1x1(inputs["enc_cv1_w"], inputs["enc_cv1_s"])      # [64,50]
    # enc dw taps in [25ch x 4 row-quarters] -> padded to 32-aligned
    e3 = inputs["enc_dw3_w"][:, 0] * inputs["enc_dw3_s"][:, None, None]
    e5 = inputs["enc_dw5_w"][:, 0] * inputs["enc_dw5_s"][:, None, None]
    w_e3 = _pad32(np.tile(e3.reshape(25, 9), (4, 1)))                 # [128,9]
    w_e5 = _pad32(np.tile(e5.reshape(25, 25), (4, 1)))                # [128,25]
    b_e3 = _pad32(np.tile(inputs["enc_dw3_b"], 4).reshape(100, 1))
    b_e5 = _pad32(np.tile(inputs["enc_dw5_b"], 4).reshape(100, 1))
    w_epx = _fold_1x1(inputs["enc_px_w"], inputs["enc_px_s"])         # [100,100]

    pa = np.zeros((128, PA_WIDTH), np.float32)
    pa[0:128, PA_CV1:PA_CV1 + 32] = w_cv1
    pa[0:64, PA_PXLO:PA_PXLO + 64] = w_px
    pa[0:64, PA_ECV1:PA_ECV1 + 50] = w_ecv1
    pa[0:100, PA_EPXA:PA_EPXA + 100] = w_epx
    # K-split weights replicated at bases 0/32/64 so lhsT base partition
    # matches the packed rhs slab's base partition
    for b0 in (0, 32, 64):
        pa[b0:b0 + 32, PA_PXHI:PA_PXHI + 64] = w_px[32:64]
        pa[b0:b0 + 25, PA_EPXB:PA_EPXB + 100] = w_epx[50:75]
        pa[b0:b0 + 25, PA_EPXC:PA_EPXC + 100] = w_epx[75:100]
    pa[0:128, PA_ID:PA_ID + 128] = np.eye(128)
    for s in range(4):
        for k in range(25):
            pa[4 * k + s, PA_ESEL + s] = 1.0                          # esel
            pa[s, PA_ESELT + 4 * k + s] = 1.0                         # esel4T
    pa[0, PA_ONES:PA_ONES + 32] = 1.0
    pa[0, PA_EONES:PA_EONES + 50] = 1.0
    d["packa"] = pa.astype(nbf)

    pb = np.zeros((128, PB_WIDTH), np.float32)
    pb[:, PB_WDWP:PB_WDWP + 25] = w_dwp
    pb[:, PB_WE3:PB_WE3 + 9] = w_e3
    pb[:, PB_WE5:PB_WE5 + 25] = w_e5
    pb[:, PB_BDWP:PB_BDWP + 1] = b_dwp
    pb[:, PB_BE3:PB_BE3 + 1] = b_e3
    pb[:, PB_BE5:PB_BE5 + 1] = b_e5
    pb[0:32, PB_BCV1:PB_BCV1 + 1] = inputs["comp_cv1_b"].reshape(32, 1)
    pb[0:64, PB_BPX:PB_BPX + 1] = inputs["comp_px_b"].reshape(64, 1)
    pb[0:50, PB_BECV1:PB_BECV1 + 1] = inputs["enc_cv1_b"].reshape(50, 1)
    pb[0:100, PB_BEPX:PB_BEPX + 1] = inputs["enc_px_b"].reshape(100, 1)
    d["packb"] = pb

    # wdiag [128, 30*128]: diagonal tap weights for PE-side dw taps
    # cols 0..14  -> comp taps 10..24 (w_dwp)
    # cols 15..29 -> enc dw5 taps 10..24 (w_e5)
    wd = np.zeros((128, 30 * 128), np.float32)
    for i in range(15):
        np.fill_diagonal(wd[:, 128 * i:128 * (i + 1)], w_dwp[:, 10 + i])
        np.fill_diagonal(wd[:, 128 * (15 + i):128 * (16 + i)], w_e5[:, 10 + i])
    d["wdiag"] = wd.astype(nbf)

    # repl [128, 4*128]: lhsT for the W row-replication matmul
    rp = np.zeros((128, 512), np.float32)
    for jb in range(4):
        for n in range(128):
            rho, j = divmod(n, 32)
            yl, xl = rho // 2, j // 2
            rp[64 * yl + 16 * jb + xl, 128 * jb + n] = 1.0
    d["repl"] = rp.astype(nbf)

    # sidx [128, 4*100] int16; horizontal out-of-image taps dropped (-1).
    si = np.full((128, 400), -1, np.int16)
    for n in range(128):
        rho, j = divmod(n, 32)
        yl, dy = divmod(rho, 2)
        xl, dx = divmod(j, 2)
        sn = 2 * dy + dx
        for jb in range(4):
            for cp in range(100):
                k, sc = divmod(cp, 4)
                if sc != sn:
                    continue
                p, q = divmod(k, 5)
                if not (0 <= 16 * jb + xl + q - 2 < 64):
                    continue
                si[n, 100 * jb + cp] = 120 * jb + 20 * (yl + p) + (xl + q)
    d["sidx"] = si
    return d


def _host_shard(X, core):
    b, ri = divmod(core, 4)
    r0 = 16 * ri - 4
    xs = np.zeros((C, XR, W), np.float32)
    lo, hi = max(0, r0), min(H, r0 + XR)
    xs[:, lo - r0 : hi - r0, :] = X[b, :, lo:hi, :]
    mrow = np.zeros((1, XR, W), np.float32)
    for r in range(XR):
        if not (0 <= r0 + r < H):
            mrow[0, r, :] = NEG
    emask = np.zeros((1, 20, W), np.float32)
    for r in range(20):
        if not (0 <= (16 * ri - 2) + r < H):
            emask[0, r, :] = NEG
    xsb = xs.astype(nbf)
    # pre-transposed X slabs, one [120, 128] per block (column-padded)
    xsp = np.zeros((C, XR, XW), nbf)
    xsp[:, :, 2 : 2 + W] = xsb
    xt = np.zeros((120, 32 * 128), nbf)
    for B in range(32):
        t, jb = divmod(B, 4)
        slab = xsp[:, 2 * t + 2 : 2 * t + 8, 16 * jb : 16 * jb + 20]
        xt[:, 128 * B : 128 * B + 128] = slab.reshape(C, 120).T
    mrowem = np.zeros((1, 2 * XR * W), np.float32)
    mrowem[0, 0 : XR * W] = mrow.reshape(XR * W)
    mrowem[0, XR * W : XR * W + 20 * W] = emask.reshape(20 * W)
    return (
        xsb.reshape(C, XR * W),
        mrowem.astype(nbf),
        xt,
    )


# ======================================================================
# device kernel
# ======================================================================

def build_kernel():
    nc = bacc.Bacc(
        "TRN2",
        target_bir_lowering=False,
        debug=False,
        enable_asserts=False,
        num_devices=NCORES,
    )

    def din(name, shape, dt):
        return nc.dram_tensor(name, list(shape), dt, kind="ExternalInput").ap()

    x_d = din("x", (128, XR * W), BF16)
    xt_d = din("xt", (120, 32 * 128), BF16)
    mrowem_d = din("mrowem", (1, 2 * XR * W), BF16)
    packa_d = din("packa", (128, PA_WIDTH), BF16)
    packb_d = din("packb", (128, PB_WIDTH), F32)
    repl_d = din("repl", (128, 512), BF16)
    wdiag_d = din("wdiag", (128, 30 * 128), BF16)
    sidx_d = din("sidx", (128, 400), I16)
    out_d = nc.dram_tensor("out", [128, 32 * 128], BF16, kind="ExternalOutput").ap()

    FS = 5 * XW                    # comp tap width 340
    FS2 = 4 * XW                   # enc tap width 272

    with tile.TileContext(nc) as tc, ExitStack() as ctx:
        cpool = ctx.enter_context(tc.tile_pool(name="consts", bufs=1))
        work = ctx.enter_context(tc.tile_pool(name="work", bufs=1))
        psF_cm = tc.tile_pool(name="psF", bufs=2, space="PSUM")
        psF = psF_cm.__enter__()
        psS_cm = tc.tile_pool(name="psS", bufs=1, space="PSUM")
        psS = psS_cm.__enter__()
        spool = ctx.enter_context(tc.tile_pool(name="stage", bufs=3))

        nc.gpsimd.load_library(library_config.local_scatter)

        packa = cpool.tile([128, PA_WIDTH], BF16, tag="packa")
        packb = cpool.tile([128, PB_WIDTH], F32, tag="packb")
        mrowem = cpool.tile([1, 2 * XR * W], BF16, tag="mrowem")
        xb = cpool.tile([128, XR * W], BF16, tag="x")
        xt = cpool.tile([120, 32 * 128], BF16, tag="xt")
        sidx = cpool.tile([128, 400], I16, tag="sidx")
        repl = cpool.tile([128, 512], BF16, tag="repl")
        wdiag = cpool.tile([128, 30 * 128], BF16, tag="wdiag")

        # sync queue: x first, then xt; scalar queue: weights + masks
        nc.sync.dma_start(xb[:], x_d)
        nc.scalar.dma_start(packa[:], packa_d)
        nc.scalar.dma_start(mrowem[:], mrowem_d)
        nc.sync.dma_start(xt[:], xt_d)
        nc.scalar.dma_start(packb[:], packb_d)
        nc.gpsimd.dma_start(sidx[:], sidx_d)
        nc.gpsimd.dma_start(repl[:], repl_d)
        nc.scalar.dma_start(wdiag[:], wdiag_d)

        w_cv1 = packa[0:128, PA_CV1:PA_CV1 + 32]
        w_px = packa[0:64, PA_PXLO:PA_PXLO + 64]
        w_ecv1 = packa[0:64, PA_ECV1:PA_ECV1 + 50]
        w_epx100 = packa[0:100, PA_EPXA:PA_EPXA + 100]
        ident = packa[0:128, PA_ID:PA_ID + 128]
        i100 = packa[0:100, PA_ID:PA_ID + 100]
        esel = packa[0:100, PA_ESEL:PA_ESEL + 4]
        esel4t = packa[0:4, PA_ESELT:PA_ESELT + 100]
        ones1 = packa[0:1, PA_ONES:PA_ONES + 32]
        eones = packa[0:1, PA_EONES:PA_EONES + 50]
        w_dwp = packb[0:128, PB_WDWP:PB_WDWP + 25]
        w_e3 = packb[0:128, PB_WE3:PB_WE3 + 9]
        w_e5 = packb[0:128, PB_WE5:PB_WE5 + 25]
        b_dwp = packb[0:128, PB_BDWP:PB_BDWP + 1]
        b_e3 = packb[0:128, PB_BE3:PB_BE3 + 1]
        b_e5 = packb[0:128, PB_BE5:PB_BE5 + 1]
        b_cv1 = packb[0:32, PB_BCV1:PB_BCV1 + 1]
        b_px = packb[0:64, PB_BPX:PB_BPX + 1]
        b_ecv1 = packb[0:50, PB_BECV1:PB_BECV1 + 1]
        b_epx = packb[0:100, PB_BEPX:PB_BEPX + 1]

        def pxhi(b0):
            return packa[b0:b0 + 32, PA_PXHI:PA_PXHI + 64]

        def epxb(b0):
            return packa[b0:b0 + 25, PA_EPXB:PA_EPXB + 100]

        def epxc(b0):
            return packa[b0:b0 + 25, PA_EPXC:PA_EPXC + 100]

        # ---- working tensors
        x12 = work.tile([64, XR * W], BF16)          # x1 (0:32) + x2 (32:64)
        enc_cat = work.tile([100, 16 * W], BF16)     # e1c + enc dw outs
        x1p = work.tile([128, 9 * XW + 8], BF16)     # packed x1 (68-pitch)
        x2p = work.tile([128, FS], BF16)             # comp dw out (packed)
        enc_in = work.tile([64, 20 * W], BF16)       # px out
        e1c = work.tile([50, 20 * W], BF16)          # enc cv1 out
        e3p = work.tile([128, 8 * XW + 8], BF16)     # enc dw3 slab
        e5p = work.tile([128, 8 * XW + 8], BF16)     # enc dw5 slab
        e3sil = work.tile([128, FS2], BF16)          # enc dw3 out (packed)
        e5sil = work.tile([128, FS2], BF16)          # enc dw5 out (packed)
        wts = work.tile([100, 1024], BF16)           # silu(enc px), transposed
        ev = work.tile([100, 1024], BF16)            # exp(wts)
        rv = work.tile([4, 1024], BF16)              # 1/colsum per subpixel
        wnorm = work.tile([100, 1024], BF16)         # softmaxed weights [sk,pix]
        # wcat_t: per-t transposed weights [pix-in-rowpair, sk]
        cA = work.tile([128, FS], BF16)
        cB = work.tile([128, FS], BF16)
        cC = work.tile([128, FS], BF16)
        cD = work.tile([128, FS], BF16)
        eA = work.tile([128, FS2], BF16)
        eB = work.tile([128, FS2], BF16)
        eC = work.tile([128, FS2], BF16)
        eF = work.tile([128, FS2], BF16)
        eG = work.tile([128, FS2], BF16)

        # warmup: trigger local_scatter ucode load early
        warm = work.tile([16, 16], BF16)
        nc.gpsimd.local_scatter(
            warm[:], packa[0:16, 0:2], sidx[:][0:16, 0:2],
            channels=16, num_elems=16, num_idxs=2,
        )
        # warmup: force silu act-table loads while inputs stream in
        wsin = work.tile([1, 2], BF16)
        wsout = work.tile([1, 2], BF16)
        nc.vector.memset(wsin[:], 0.0)
        nc.scalar.activation(wsout[:], wsin[:], AF.Silu)

        x123 = x12[:].rearrange("p (r c) -> p r c", c=W)
        enc_cat3 = enc_cat[:].rearrange("p (r c) -> p r c", c=W)
        e1c3 = e1c[:].rearrange("p (r c) -> p r c", c=W)
        x1p3 = x1p[:, 0 : 9 * XW].rearrange("p (r c) -> p r c", c=XW)
        e3p3 = e3p[:, 0 : 8 * XW].rearrange("p (r c) -> p r c", c=XW)
        e5p3 = e5p[:, 0 : 8 * XW].rearrange("p (r c) -> p r c", c=XW)

        # zero slab pad columns + pad partitions (25:32 per quarter)
        nc.vector.memset(x1p[:, 9 * XW : 9 * XW + 8], 0.0)
        nc.vector.memset(x1p3[:, :, 0:2], 0.0)
        nc.vector.memset(x1p3[:, :, 66:68], 0.0)
        nc.vector.memset(e3p[:], 0.0)
        nc.vector.memset(e5p[:], 0.0)

        # ---- comp cv1 + interleaved x1p packing
        for ch in range(3):
            ps = psF.tile([32, 512], F32, tag="convps")
            nc.tensor.matmul(
                ps[:], w_cv1, xb[:, 512 * ch : 512 * (ch + 1)],
                start=True, stop=False,
            )
            nc.tensor.matmul(
                ps[:], ones1, mrowem[0:1, 512 * ch : 512 * (ch + 1)],
                start=False, stop=True,
            )
            nc.scalar.activation(
                x12[0:32, 512 * ch : 512 * (ch + 1)], ps[:],
                AF.Silu, bias=b_cv1,
            )
            # pack row-groups as soon as their rows exist:
            # g0 needs rows 0:9 (ch0+ch1), g1 rows 5:14, g2 rows 10:19,
            # g3 rows 15:24 (ch2)
            packs = {1: (0, 1), 2: (2, 3)}.get(ch, ())
            for g in packs:
                eng = (nc.gpsimd, nc.gpsimd, nc.sync, nc.sync)[g]
                eng.dma_start(
                    x1p3[32 * g : 32 * g + 32, 0:9, 2 : 2 + W],
                    x123[0:32, 5 * g : 5 * g + 9, :],
                )

        # ---- comp dw: 25 unified 5x5 taps, DVE (15) + Pool (10)
        def tap(eng, acc, slab, wsel, taps, offs, first):
            for i, t in enumerate(taps):
                sv = slab[:, offs[t] : offs[t] + (FS if slab is x1p else FS2)]
                if i == 0 and first:
                    eng.tensor_scalar(acc, sv, wsel[:, t : t + 1], None, OP.mult)
                else:
                    eng.scalar_tensor_tensor(
                        acc, sv, wsel[:, t : t + 1], acc, OP.mult, OP.add
                    )

        off5c = [(t // 5) * XW + (t % 5) for t in range(25)]
        psd = psF.tile([128, FS], F32, tag="dwps")
        for i in range(15):
            t = 10 + i
            nc.tensor.matmul(
                psd[:], wdiag[:, 128 * i : 128 * (i + 1)],
                x1p[:, off5c[t] : off5c[t] + FS],
                start=(i == 0), stop=(i == 14),
            )
        for i in range(5):
            tap(nc.vector, cA[:], x1p, w_dwp, [2 * i], off5c, i == 0)
            tap(nc.vector, cB[:], x1p, w_dwp, [2 * i + 1], off5c, i == 0)
        nc.vector.tensor_copy(cC[:], psd[:])
        nc.vector.tensor_tensor(cA[:], cA[:], cB[:], OP.add)
        nc.vector.tensor_tensor(cA[:], cA[:], cC[:], OP.add)
        nc.scalar.activation(x2p[:], cA[:], AF.Silu, bias=b_dwp)

        # unpack x2p into x12 rows 32:64 (image rows 2..21)
        for g in range(4):
            (nc.sync, nc.sync, nc.gpsimd, nc.gpsimd)[g].dma_start(
                x123[32:64, 2 + 5 * g : 7 + 5 * g, :],
                x2p[32 * g : 32 * g + 32, :].rearrange(
                    "p (r c) -> p r c", c=XW
                )[:, 0:5, 0:W],
            )

        # ---- comp px: 1x1 conv 64->64 (+ SiLU)
        for r0, nr in ((0, 8), (8, 8), (16, 4)):
            ps = psF.tile([64, 512], F32, tag="convps")
            nc.tensor.matmul(
                ps[:, : nr * W], w_px,
                x12[0:64, (2 + r0) * W : (2 + r0 + nr) * W],
                start=True, stop=True,
            )
            nc.scalar.activation(
                enc_in[0:64, r0 * W : (r0 + nr) * W], ps[:, : nr * W],
                AF.Silu, bias=b_px,
            )

        # ---- enc cv1: 1x1 conv 64->50 (+ SiLU, mask row rides K=65) and
        # interleaved e3p/e5p packing (quarter q needs e1c rows 4q:4q+8)
        epack_engs = iter((nc.sync, nc.gpsimd, nc.sync, nc.gpsimd,
                           nc.sync, nc.gpsimd, nc.sync, nc.sync))
        for ci, (r0, nr) in enumerate(((0, 8), (8, 8), (16, 4))):
            ps = psF.tile([50, 512], F32, tag="convps")
            nc.tensor.matmul(
                ps[:, : nr * W], w_ecv1,
                enc_in[0:64, r0 * W : (r0 + nr) * W],
                start=True, stop=False,
            )
            nc.tensor.matmul(
                ps[:, : nr * W], eones,
                mrowem[0:1, XR * W + r0 * W : XR * W + (r0 + nr) * W],
                start=False, stop=True,
            )
            nc.scalar.activation(
                e1c[0:50, r0 * W : (r0 + nr) * W], ps[:, : nr * W],
                AF.Silu, bias=b_ecv1,
            )
            qs = {0: (0,), 1: (1, 2), 2: (3,)}[ci]
            for q in qs:
                next(epack_engs).dma_start(
                    e3p3[32 * q : 32 * q + 25, 0:8, 2 : 2 + W],
                    e1c3[0:25, 4 * q : 4 * q + 8, :],
                )
                next(epack_engs).dma_start(
                    e5p3[32 * q : 32 * q + 25, 0:8, 2 : 2 + W],
                    e1c3[25:50, 4 * q : 4 * q + 8, :],
                )

        # ---- enc dw: dw5 25 taps + dw3 9 taps, split DVE/Pool
        off5 = [(t // 5) * XW + (t % 5) for t in range(25)]
        off3 = [((t // 3) + 1) * XW + (t % 3) + 1 for t in range(9)]
        pse = psF.tile([128, FS2], F32, tag="dwps")
        for i in range(15):
            t = 10 + i
            nc.tensor.matmul(
                pse[:], wdiag[:, 128 * (15 + i) : 128 * (16 + i)],
                e5p[:, off5[t] : off5[t] + FS2],
                start=(i == 0), stop=(i == 14),
            )
        for i in range(5):
            tap(nc.vector, eA[:], e5p, w_e5, [2 * i], off5, i == 0)
            tap(nc.vector, eB[:], e5p, w_e5, [2 * i + 1], off5, i == 0)
        for i in range(5):
            tap(nc.vector, eF[:], e3p, w_e3, [i], off3, i == 0)
            if i < 4:
                tap(nc.vector, eG[:], e3p, w_e3, [5 + i], off3, i == 0)
        nc.vector.tensor_copy(eC[:], pse[:])
        nc.vector.tensor_tensor(eA[:], eA[:], eB[:], OP.add)
        nc.vector.tensor_tensor(eA[:], eA[:], eC[:], OP.add)
        nc.vector.tensor_tensor(eF[:], eF[:], eG[:], OP.add)
        nc.scalar.activation(e5sil[:], eA[:], AF.Silu, bias=b_e5)
        nc.scalar.activation(e3sil[:], eF[:], AF.Silu, bias=b_e3)

        # ---- assemble enc_cat [100, 16W]: e1c interior + enc dw outs
        nc.sync.dma_start(enc_cat[0:50, :], e1c[0:50, 2 * W : 18 * W])
        for q in range(4):
            (nc.gpsimd if q < 2 else nc.sync).dma_start(
                enc_cat3[50:75, 4 * q : 4 * q + 4, :],
                e3sil[32 * q : 32 * q + 25, :].rearrange(
                    "p (r c) -> p r c", c=XW
                )[:, 0:4, 0:W],
            )
            nc.sync.dma_start(
                enc_cat3[75:100, 4 * q : 4 * q + 4, :],
                e5sil[32 * q : 32 * q + 25, :].rearrange(
                    "p (r c) -> p r c", c=XW
                )[:, 0:4, 0:W],
            )

        # ---- enc px transposed: WT[sk, pix], 2 chunks of 512
        for hc in range(2):
            ps = psF.tile([100, 512], F32, tag="convps")
            nc.tensor.matmul(
                ps[:], w_epx100, enc_cat[0:100, 512 * hc : 512 * (hc + 1)],
                start=True, stop=True,
            )
            nc.scalar.activation(
                wts[:, 512 * hc : 512 * (hc + 1)], ps[:], AF.Silu, bias=b_epx
            )

        # ---- softmax over 25 taps (transposed); single exp op so the
        # Act queue never alternates Silu/Exp tables
        nc.scalar.activation(ev[:], wts[:], AF.Exp)
        for hc in range(2):
            sl = slice(512 * hc, 512 * (hc + 1))
            ssum = psS.tile([4, 512], F32, tag="ssum")
            nc.tensor.matmul(ssum[:], esel, ev[:, sl], start=True, stop=True)
            with nc.allow_low_precision(reason="bf16 softmax denominators"):
                nc.vector.reciprocal(rv[:, sl], ssum[:])
            rrep = psS.tile([100, 512], F32, tag="rrep")
            nc.tensor.matmul(rrep[:], esel4t, rv[:, sl], start=True, stop=True)
            nc.vector.tensor_tensor(wnorm[:, sl], ev[:, sl], rrep[:], OP.mult)

        psS_cm.__exit__(None, None, None)
        psF_cm.__exit__(None, None, None)
        psB = ctx.enter_context(tc.tile_pool(name="psB", bufs=2, space="PSUM"))
        psC = ctx.enter_context(tc.tile_pool(name="psC", bufs=2, space="PSUM"))
        psO = ctx.enter_context(tc.tile_pool(name="psO", bufs=2, space="PSUM"))
        psW = ctx.enter_context(tc.tile_pool(name="psW", bufs=2, space="PSUM"))

        # ---- backend t-loop: replicate+transpose matmul -> scatter ->
        # PE transpose -> big matmul -> bf16 out
        for t in range(8):
            psw = psW.tile([128, 100], BF16, tag="wcatps")
            nc.tensor.transpose(
                psw[:], wnorm[:, 128 * t : 128 * t + 128], i100,
            )
            wcat = spool.tile([128, 100], BF16, tag="wcat")
            if t % 2 == 0:
                nc.scalar.copy(wcat[:], psw[:])
            else:
                nc.vector.tensor_copy(wcat[:], psw[:])
            psb = psB.tile([128, 400], F32, tag="dallps")
            for jb in range(4):
                nc.tensor.matmul(
                    psb[:, 100 * jb : 100 * jb + 100],
                    repl[:, 128 * jb : 128 * jb + 128], wcat[:],
                    start=True, stop=True,
                )
            dall = spool.tile([128, 400], BF16, tag="dall")
            nc.scalar.copy(dall[:], psb[:])

            b4t = spool.tile([128, 480], BF16, tag="b4t")
            nc.gpsimd.local_scatter(
                b4t[:], dall[:], sidx[:],
                channels=128, num_elems=480, num_idxs=400,
            )

            psc = psC.tile([120, 512], BF16, tag="b4ps")
            for jb in range(4):
                nc.tensor.transpose(
                    psc[:, 128 * jb : 128 * jb + 128],
                    b4t[:, 120 * jb : 120 * jb + 120],
                    ident,
                )
            b4 = spool.tile([120, 512], BF16, tag="b4")
            nc.vector.tensor_copy(b4[:], psc[:])

            pso = psO.tile([128, 512], F32, tag="outps")
            for jb in range(4):
                nc.tensor.matmul(
                    pso[:, 128 * jb : 128 * jb + 128],
                    xt[:, 512 * t + 128 * jb : 512 * t + 128 * jb + 128],
                    b4[:, 128 * jb : 128 * jb + 128],
                    start=True, stop=True,
                )
            stg = spool.tile([128, 512], BF16, tag="ostage")
            if t % 2 == 0:
                nc.vector.tensor_copy(stg[:], pso[:])
            else:
                nc.scalar.copy(stg[:], pso[:])
            nc.sync.dma_start(
                out_d[:, 512 * t : 512 * (t + 1)], stg[:]
            )

    nc.compile()
    return nc


_NC_CACHE = None


def _get_nc():
    global _NC_CACHE
    if _NC_CACHE is None:
        _NC_CACHE = build_kernel()
    return _NC_CACHE


def kernel(**inputs) -> np.ndarray:
    X = np.asarray(inputs["X"], np.float32)
    consts = _host_consts(
        {k: np.asarray(v, np.float32) for k, v in inputs.items() if k != "X"}
    )
    in_maps = []
    for core in range(NCORES):
        xs, mrowem, xt = _host_shard(X, core)
        m = dict(consts)
        m["x"] = xs
        m["mrowem"] = mrowem
        m["xt"] = xt
        in_maps.append(m)

    nc = _get_nc()
    res = run_bass_kernel_spmd(nc, in_maps, core_ids=list(range(NCORES)))
    out = np.zeros((2, C, 128, 128), np.float32)
    for core in range(NCORES):
        b, ri = divmod(core, 4)
        # stg layout per t: [c, (jb, rho, j32)] -> rows 4t+rho, cols 32jb+j32
        o = res.results[core]["out"].astype(np.float32)
        o = o.reshape(C, 8, 4, 4, 32).transpose(0, 1, 3, 2, 4).reshape(C, 32, 128)
        out[b, :, 32 * ri : 32 * ri + 32, :] = o
    return out


if __name__ == "__main__":
    print("smoke build only")
    build_kernel()
    print("build ok")
